# revision 10
# baseline (speedup 1.0000x reference)
"""DeltaSynapse kernel for Trainium2 (8 NeuronCores, SPMD).

Reference computation:
    Xpre[b,e,o] = sum_d delaymap[d,e,o] * Xd[d,b,e]
    I[b,o]      = sum_e (signs*W)[e,o] * Xpre[b,e,o]

Folded:  I[b,o] = sum_{d,e} (delaymap[d,e,o] * Weff[e,o]) * Xd[d,b,e]
i.e. a sum of D matmuls  I += Xd[d] @ (delaymap[d] . Weff).

Sharding: column-shard the post dim `o` across 8 cores (256 cols each).
Each core reads its own slice of delaymap/W/signs plus a replicated Xd
(~21 MiB/core) and writes a disjoint [16, 256] output slice -> host
concat. Memory-bound: per-core roofline ~ 21 MiB at ~420 GB/s ~ 52 us.

Pipeline per slab (1 MiB of delaymap = one e-chunk of 128):
  DMA slab -> elementwise (delaymap . Weff) split DVE (d 0..4) /
  GpSimd (d 5..7) -> 8 accumulating PE matmuls (fp32r, full rate).
"""

import numpy as np

D, B, N = 8, 16, 2048
NCORES = 8
P = 128                 # SBUF partitions / matmul contraction tile
OSH = N // NCORES       # per-core post-dim shard = 256
NCH = N // P            # e-chunks = 16
DSPLIT = 5              # d-planes multiplied on DVE; rest on GpSimd
WS_HEAD = 2             # e-chunks of W/signs in the head DMA

_prog_cache = {}


def _build_program():
    from concourse import bacc, tile
    from concourse import mybir

    f32 = mybir.dt.float32
    f32r = mybir.dt.float32r

    nc = bacc.Bacc()
    # Host-prepared layouts (see kernel() below):
    #   dm  : [NCH, P, D, OSH]          delaymap slice, e = c*128+p
    #   wsa : [P, 2, WS_HEAD, OSH]      W/signs chunks 0..WS_HEAD-1
    #   wsb : [P, 2, NCH-WS_HEAD, OSH]  W/signs chunks WS_HEAD..
    #   xd  : [P, NCH, D, B]            Xd transposed (replicated)
    dm = nc.dram_tensor("dm", [NCH, P, D, OSH], f32, kind="ExternalInput")
    wsa = nc.dram_tensor("wsa", [P, 2, WS_HEAD, OSH], f32, kind="ExternalInput")
    wsb = nc.dram_tensor(
        "wsb", [P, 2, NCH - WS_HEAD, OSH], f32, kind="ExternalInput"
    )
    xd = nc.dram_tensor("xd", [P, NCH, D, B], f32, kind="ExternalInput")
    out = nc.dram_tensor("out", [B, OSH], f32, kind="ExternalOutput")

    with tile.TileContext(nc) as tc:
        with (
            tc.tile_pool(name="const", bufs=1) as cpool,
            tc.tile_pool(name="dm", bufs=6) as dmpool,
            tc.tile_pool(name="wda", bufs=4) as wdapool,
            tc.tile_pool(name="wdb", bufs=4) as wdbpool,
            tc.tile_pool(name="psum", bufs=1, space="PSUM") as ppool,
            tc.tile_pool(name="outp", bufs=1) as opool,
        ):
            wsa_t = cpool.tile([P, 2, WS_HEAD, OSH], f32)
            wsb_t = cpool.tile([P, 2, NCH - WS_HEAD, OSH], f32)
            weff = cpool.tile([P, NCH, OSH], f32)
            xd_t = cpool.tile([P, NCH, D, B], f32)
            xd_r = cpool.tile([P, NCH, D, B], f32r)

            # DMA issue order on the sync queue: small ws head first, then
            # the first delaymap slabs, xd, the ws tail, remaining slabs.
            nc.sync.dma_start(wsa_t[:], wsa[:])
            dm_tiles = []
            for c in range(NCH):
                t = dmpool.tile([P, D, OSH], f32, tag="dmslab")
                dm_tiles.append(t)
            nc.sync.dma_start(dm_tiles[0][:], dm[0])
            nc.sync.dma_start(xd_t[:], xd[:])
            nc.sync.dma_start(dm_tiles[1][:], dm[1])
            nc.sync.dma_start(wsb_t[:], wsb[:])
            for c in range(2, NCH):
                nc.sync.dma_start(dm_tiles[c][:], dm[c])

            # Weff = W * signs (gpsimd for the head so DVE stays free)
            nc.gpsimd.tensor_mul(
                weff[:, 0:WS_HEAD, :], wsa_t[:, 0], wsa_t[:, 1]
            )
            nc.vector.tensor_mul(
                weff[:, WS_HEAD:NCH, :], wsb_t[:, 0], wsb_t[:, 1]
            )
            nc.vector.tensor_copy(xd_r[:], xd_t[:])

            psum = ppool.tile([B, OSH], f32)
            n_mm = NCH * D
            i = 0
            for c in range(NCH):
                dm_t = dm_tiles[c]
                wd_a = wdapool.tile([P, DSPLIT, OSH], f32r)
                wd_b = wdbpool.tile([P, D - DSPLIT, OSH], f32r)
                weff_c = weff[:, c, :].unsqueeze(1)
                nc.vector.tensor_mul(
                    wd_a[:],
                    dm_t[:, 0:DSPLIT, :],
                    weff_c.broadcast_to([P, DSPLIT, OSH]),
                )
                nc.gpsimd.tensor_mul(
                    wd_b[:],
                    dm_t[:, DSPLIT:D, :],
                    weff_c.broadcast_to([P, D - DSPLIT, OSH]),
                )
                for d in range(D):
                    src = wd_a[:, d, :] if d < DSPLIT else wd_b[:, d - DSPLIT, :]
                    nc.tensor.matmul(
                        psum[:],
                        xd_r[:, c, d, :],
                        src,
                        start=(i == 0),
                        stop=(i == n_mm - 1),
                    )
                    i += 1

            out_t = opool.tile([B, OSH], f32)
            nc.scalar.copy(out_t[:], psum[:])
            nc.sync.dma_start(out[:], out_t[:])

    nc.compile()
    return nc


def _get_program():
    if "nc" not in _prog_cache:
        _prog_cache["nc"] = _build_program()
    return _prog_cache["nc"]


def _shard_inputs(Xd, delaymap, W, signs):
    """Pure layout permutation/slicing -> per-core input maps."""
    Xd = np.ascontiguousarray(np.asarray(Xd, dtype=np.float32))
    delaymap = np.asarray(delaymap, dtype=np.float32)
    W = np.asarray(W, dtype=np.float32)
    signs = np.asarray(signs, dtype=np.float32)

    # Xd [D,B,N] -> [P, NCH, D, B] (replicated to every core)
    xdT = np.ascontiguousarray(Xd.reshape(D, B, NCH, P).transpose(3, 2, 0, 1))

    in_maps = []
    for k in range(NCORES):
        osl = slice(k * OSH, (k + 1) * OSH)
        # delaymap [D,N,OSH] -> [NCH, P, D, OSH]
        dmk = np.ascontiguousarray(
            delaymap[:, :, osl].reshape(D, NCH, P, OSH).transpose(1, 2, 0, 3)
        )
        # W/signs [N,OSH] -> [P, 2, NCH, OSH], split into head/tail chunks
        wk = W[:, osl].reshape(NCH, P, OSH).transpose(1, 0, 2)
        sk = signs[:, osl].reshape(NCH, P, OSH).transpose(1, 0, 2)
        ws = np.stack([wk, sk], axis=1)  # [P, 2, NCH, OSH]
        wsa = np.ascontiguousarray(ws[:, :, :WS_HEAD])
        wsb = np.ascontiguousarray(ws[:, :, WS_HEAD:])
        in_maps.append({"dm": dmk, "wsa": wsa, "wsb": wsb, "xd": xdT})
    return in_maps


def _run(in_maps, trace=False, **kw):
    from concourse.bass_utils import run_bass_kernel_spmd

    nc = _get_program()
    return run_bass_kernel_spmd(nc, in_maps, list(range(NCORES)), trace=trace, **kw)


def kernel(Xd, X, delaymap, W, signs):
    in_maps = _shard_inputs(Xd, delaymap, W, signs)
    res = _run(in_maps)
    return np.concatenate(
        [res.results[k]["out"] for k in range(NCORES)], axis=1
    )


# revision 12
# speedup vs baseline: 1.1803x; 1.1803x over previous
"""DeltaSynapse kernel for Trainium2 (8 NeuronCores, SPMD).

Reference computation:
    Xpre[b,e,o] = sum_d delaymap[d,e,o] * Xd[d,b,e]
    I[b,o]      = sum_e (signs*W)[e,o] * Xpre[b,e,o]

Folded:  I[b,o] = sum_{d,e} (delaymap[d,e,o] * Weff[e,o]) * Xd[d,b,e]
i.e. a sum of D matmuls  I += Xd[d] @ (delaymap[d] . Weff).

Sharding: column-shard the post dim `o` across 8 cores (256 cols each).
Each core reads its own slice of delaymap/W/signs plus a replicated Xd
(~21 MiB/core of fp32 HBM reads) and writes a disjoint [16, 256] output
slice -> host concat. Memory-bound: roofline ~ 21 MiB / ~420 GB/s.

On-chip dtype: fp16. delaymap is one-hot (0/1 -> exact in fp16); W/Xd
lose only 2^-11 rel. SWDGE DMA casts fp32->fp16 in the datapath, so
HBM reads stay fp32 (full bytes) while SBUF tiles halve and the DVE
multiply runs in 2x mode. PE runs fp16 at full rate (1 cyc/row).
"""

import numpy as np

D, B, N = 8, 16, 2048
NCORES = 8
P = 128                 # SBUF partitions / matmul contraction tile
OSH = N // NCORES       # per-core post-dim shard = 256
NCH = N // P            # e-chunks = 16
CGRP = 2                # e-chunks per DMA slab (2 MiB HBM reads)
NSLAB = NCH // CGRP
WS_HEAD = 2             # e-chunks of W/signs in the head DMA

_prog_cache = {}


def _build_program():
    from concourse import bacc, tile
    from concourse import mybir

    f32 = mybir.dt.float32
    f16 = mybir.dt.float16

    nc = bacc.Bacc()
    # Host-prepared layouts (see kernel() below), all fp32 in HBM:
    #   dm  : [NSLAB, P, CGRP, D, OSH]  delaymap slice, e=(s*CGRP+c2)*128+p
    #   wsa : [P, 2, WS_HEAD, OSH]      W/signs chunks 0..WS_HEAD-1
    #   wsb : [P, 2, NCH-WS_HEAD, OSH]  W/signs remaining chunks
    #   xd  : [P, NCH, D, B]            Xd transposed (replicated)
    dm = nc.dram_tensor("dm", [NSLAB, P, CGRP, D, OSH], f32, kind="ExternalInput")
    wsa = nc.dram_tensor("wsa", [P, 2, WS_HEAD, OSH], f32, kind="ExternalInput")
    wsb = nc.dram_tensor(
        "wsb", [P, 2, NCH - WS_HEAD, OSH], f32, kind="ExternalInput"
    )
    xd = nc.dram_tensor("xd", [P, NCH, D, B], f32, kind="ExternalInput")
    out = nc.dram_tensor("out", [B, OSH], f32, kind="ExternalOutput")

    with tile.TileContext(nc) as tc:
        with (
            tc.tile_pool(name="const", bufs=1) as cpool,
            tc.tile_pool(name="dm", bufs=4) as dmpool,
            tc.tile_pool(name="wd", bufs=6) as wdpool,
            tc.tile_pool(name="psum", bufs=1, space="PSUM") as ppool,
            tc.tile_pool(name="outp", bufs=1) as opool,
        ):
            wsa_t = cpool.tile([P, 2, WS_HEAD, OSH], f16)
            wsb_t = cpool.tile([P, 2, NCH - WS_HEAD, OSH], f16)
            weff = cpool.tile([P, NCH, OSH], f16)
            xd_t = cpool.tile([P, NCH, D, B], f16)

            # SWDGE (gpsimd) DMAs cast fp32->fp16 in the datapath.
            # Issue order: small ws head, first slab, xd, ws tail, rest.
            nc.gpsimd.dma_start(wsa_t[:], wsa[:])
            dm_tiles = [
                dmpool.tile([P, CGRP, D, OSH], f16, tag="dmslab", name=f"dm{s}")
                for s in range(NSLAB)
            ]
            nc.gpsimd.dma_start(dm_tiles[0][:], dm[0])
            nc.gpsimd.dma_start(xd_t[:], xd[:])
            nc.gpsimd.dma_start(dm_tiles[1][:], dm[1])
            nc.gpsimd.dma_start(wsb_t[:], wsb[:])
            for s in range(2, NSLAB):
                nc.gpsimd.dma_start(dm_tiles[s][:], dm[s])

            nc.vector.tensor_mul(
                weff[:, 0:WS_HEAD, :], wsa_t[:, 0], wsa_t[:, 1]
            )
            nc.vector.tensor_mul(
                weff[:, WS_HEAD:NCH, :], wsb_t[:, 0], wsb_t[:, 1]
            )

            psum = ppool.tile([B, OSH], f32)
            n_mm = NCH * D
            i = 0
            for s in range(NSLAB):
                dm_t = dm_tiles[s]
                for c2 in range(CGRP):
                    c = s * CGRP + c2
                    wd_t = wdpool.tile([P, D, OSH], f16, tag="wd")
                    nc.vector.tensor_mul(
                        wd_t[:],
                        dm_t[:, c2],
                        weff[:, c, :].unsqueeze(1).broadcast_to([P, D, OSH]),
                    )
                    for d in range(D):
                        nc.tensor.matmul(
                            psum[:],
                            xd_t[:, c, d, :],
                            wd_t[:, d, :],
                            start=(i == 0),
                            stop=(i == n_mm - 1),
                        )
                        i += 1

            out_t = opool.tile([B, OSH], f32)
            nc.scalar.copy(out_t[:], psum[:])
            nc.sync.dma_start(out[:], out_t[:])

    nc.compile()
    return nc


def _get_program():
    if "nc" not in _prog_cache:
        _prog_cache["nc"] = _build_program()
    return _prog_cache["nc"]


def _shard_inputs(Xd, delaymap, W, signs):
    """Pure layout permutation/slicing -> per-core input maps."""
    Xd = np.ascontiguousarray(np.asarray(Xd, dtype=np.float32))
    delaymap = np.asarray(delaymap, dtype=np.float32)
    W = np.asarray(W, dtype=np.float32)
    signs = np.asarray(signs, dtype=np.float32)

    # Xd [D,B,N] -> [P, NCH, D, B] (replicated to every core)
    xdT = np.ascontiguousarray(Xd.reshape(D, B, NCH, P).transpose(3, 2, 0, 1))

    in_maps = []
    for k in range(NCORES):
        osl = slice(k * OSH, (k + 1) * OSH)
        # delaymap [D,N,OSH] -> [NSLAB, P, CGRP, D, OSH]
        dmk = np.ascontiguousarray(
            delaymap[:, :, osl]
            .reshape(D, NSLAB, CGRP, P, OSH)
            .transpose(1, 3, 2, 0, 4)
        )
        # W/signs [N,OSH] -> [P, 2, NCH, OSH], split into head/tail chunks
        wk = W[:, osl].reshape(NCH, P, OSH).transpose(1, 0, 2)
        sk = signs[:, osl].reshape(NCH, P, OSH).transpose(1, 0, 2)
        ws = np.stack([wk, sk], axis=1)  # [P, 2, NCH, OSH]
        wsa = np.ascontiguousarray(ws[:, :, :WS_HEAD])
        wsb = np.ascontiguousarray(ws[:, :, WS_HEAD:])
        in_maps.append({"dm": dmk, "wsa": wsa, "wsb": wsb, "xd": xdT})
    return in_maps


def _run(in_maps, trace=False, **kw):
    from concourse.bass_utils import run_bass_kernel_spmd

    nc = _get_program()
    return run_bass_kernel_spmd(nc, in_maps, list(range(NCORES)), trace=trace, **kw)


def kernel(Xd, X, delaymap, W, signs):
    in_maps = _shard_inputs(Xd, delaymap, W, signs)
    res = _run(in_maps)
    return np.concatenate(
        [res.results[k]["out"] for k in range(NCORES)], axis=1
    )
